# revision 36
# baseline (speedup 1.0000x reference)
"""Trainium2 Bass kernel for KeypointSelector:
conv3x3(384->128, pad 1) + bias + ReLU -> conv1x1(128->1) + bias + sigmoid.

Input  dino_features: (32, 64, 64, 384) f32
Output (32, 64, 64, 1) f32

Strategy: pure data parallel over batch, 4 images per core on 8 cores.
The 3x3 conv runs in fp8e4m3 with MatmulPerfMode.DoubleRow (0.5 PE
cycles per output column): the 27 contraction k-tiles (3 cin chunks x 9
taps) pack into 14 DoubleRow pairs (the 28th slot carries zero weights).

Block-interleaved input layout: each image is stored as 8 row-blocks of
[128, 1980] fp8 -- padded rows [8t, 8t+10) of chunk0|chunk1|chunk2 side
by side (660 cols each). Every conv output tile t (8 rows x 64 cols =
512 px) reads ONLY block t, so one small DMA unblocks the first matmuls
~3us earlier than whole-chunk loads, and all DoubleRow pair deltas stay
in [596, 1387] > the 526-col moving window (no overlapping reads).

The pointwise tail runs entirely off the PE:
- DVE: g = max(psum, -SX*SW*b1) * (W2*DESCALE) per channel, one
  scalar_tensor_tensor (per-partition scalar AP + stride-0 broadcast).
  Identity: sum_c W2_c*(max(conv_c, -b1_c) + b1_c) = conv1x1(relu(...)),
  with the constant sum_c W2_c*b1_c folded into the sigmoid bias.
- GpSimd: partition_all_reduce(add) sums the 128 channels.
- ACT: sigmoid with bias b2 + sum(W2*b1).
This removes the old PE 1x1 matmuls (~3.4us of PE time); the PE now runs
only the conv (14 pairs x 8 tiles x 4 images = 114688 cycles ~ 47.8us).

Cold start: w1 is split in two DMAs around image0/block0 so the first
matmul needs only 1.3us of serial DMA; image 0 uses single-tile groups
so tile 1's data cannot stall the k-loop; the PE p-state ramp is primed
with matmuls on a DVE-memset zeros tile (DVE, not ACT: the ACT-engine
memzero hides a 1.3us LoadActFuncSet on the critical path).
Tail: the last image ends with a 2-row (128 px) segment so the final
relu/reduce/sigmoid/DMA chain is short.
"""

import ml_dtypes
import numpy as np

import concourse.tile as tile
from concourse import bacc, bass_isa, mybir
from concourse.ap import AP
from concourse.bass_utils import run_bass_kernel_spmd

E4M3 = ml_dtypes.float8_e4m3  # == mybir.dt.np(float8e4)

# Geometry
B, H, W, CIN, CHID = 32, 64, 64, 384, 128
NCORES = 8
BLOC = B // NCORES  # 4 images per core
BW = W + 2  # padded row width (66)
NCHUNK = CIN // 128  # 3 cin chunks
TS = 512  # matmul free-dim tile (one PSUM bank of fp32)
ROWS_PT = 8  # output rows per tile
NT = H // ROWS_PT  # 8 tiles per image
BROWS = ROWS_PT + 2  # padded rows per block (halo)
CHW = BROWS * BW  # chunk width inside a block (660)
BLKW = NCHUNK * CHW  # block width (1980)
XFW = NT * BLKW  # flat image tile width (15840)
OUTW = H * W  # densely packed flat output row (4096)

# fp8 scaling
SX, SW = 16.0, 256.0
DESCALE = 1.0 / (SX * SW)

# In-block offset of tap t=(dy+1)*3+(dx+1): (1+dy)*BW + (1+dx).
TOFF = [0, 1, 2, BW, BW + 1, BW + 2, 2 * BW, 2 * BW + 1, 2 * BW + 2]

# k-tile pairing for DoubleRow: 27 (chunk, tap) k-tiles -> 13 full pairs
# + 1 half pair whose slot A reads (c0, t1) with zero weights (the data
# is finite fp8, so 0*x contributes 0 to PSUM). Two hardware constraints
# on the slot A -> slot B rhs delta, found empirically (the device takes
# NRT_EXEC_UNIT_UNRECOVERABLE otherwise):
#   1. delta > 526 (the 8x64 moving window span) -- no overlapping reads;
#   2. delta must be EVEN: odd deltas crash the DoubleRow dual-stream
#      ifmap fetch. TOFF parity is odd exactly for taps t%3==1 (dx=0),
#      so pairs match taps of equal dx-parity across chunks.
PAIRS = [
    (0, 0, 1, 0), (0, 2, 1, 2), (0, 3, 1, 3), (0, 1, 1, 1), (0, 4, 1, 4),
    (0, 5, 2, 0), (0, 6, 2, 2), (0, 8, 2, 3), (0, 7, 2, 1),
    (1, 5, 2, 5), (1, 6, 2, 6), (1, 8, 2, 8), (1, 7, 2, 4),
    (None, 1, 2, 7),  # slot A zero-weighted
]
NPAIR = len(PAIRS)  # 14
BASEA, DELTA = [], []
for cA, tA, cB, tB in PAIRS:
    ca = 0 if cA is None else cA
    BASEA.append(ca * CHW + TOFF[tA])
    DELTA.append(cB * CHW + TOFF[tB] - BASEA[-1])
assert all(d > 7 * BW + W and d % 2 == 0 for d in DELTA), DELTA
assert all(BASEA[k] + DELTA[k] + 7 * BW + W - 1 < BLKW for k in range(NPAIR))
_taps = sorted((c, t) for cA, tA, cB, tB in PAIRS
               for c, t in ([(cA, tA)] if cA is not None else []) + [(cB, tB)])
assert _taps == sorted((c, t) for c in range(3) for t in range(9)), _taps

W1SPLIT = 7  # w1 pairs [0, 7) land before block 0, the rest after

_CACHED = {}


def _build_bass(reps=1):
    nc = bacc.Bacc("TRN2", target_bir_lowering=False)

    f32 = mybir.dt.float32
    bf16 = mybir.dt.bfloat16
    fp8 = mybir.dt.float8e4
    DR = mybir.MatmulPerfMode.DoubleRow

    x = nc.dram_tensor("x", [BLOC, NT, 128, BLKW], fp8, kind="ExternalInput")
    w1 = nc.dram_tensor("w1", [128, NPAIR, 2, CHID], fp8, kind="ExternalInput")
    cst = nc.dram_tensor("cst", [128, 3], f32, kind="ExternalInput")
    y = nc.dram_tensor("y", [BLOC, OUTW], f32, kind="ExternalOutput")

    with tile.TileContext(nc) as tc:
        with (
            tc.tile_pool(name="consts", bufs=1) as consts,
            tc.tile_pool(name="xin", bufs=2) as xin,
            tc.tile_pool(name="gbuf", bufs=4) as gbuf,
            tc.tile_pool(name="rbuf", bufs=4) as rbuf,
            # obuf depth 4: with 2 buffers the tail sigmoids serialize on
            # the y DMA + 900ns sem-prop of the group two flushes back.
            tc.tile_pool(name="obuf", bufs=4) as obuf,
            tc.tile_pool(name="ps1", bufs=3, space="PSUM") as ps1,
            tc.tile_pool(name="psp", bufs=1, space="PSUM") as psp,
        ):
            # w1 first half on the SP queue ahead of image 0 block 0; the
            # first Ldweights needs it. Second half lands after block 0.
            w1_s = consts.tile([128, NPAIR, 2, CHID], fp8)
            nc.sync.dma_start(out=w1_s[:, 0:W1SPLIT], in_=w1[:, 0:W1SPLIT])
            # Pointwise consts (w2*DESCALE | -SX*SW*b1 | sigmoid bias) as ONE
            # tiny DMA, emitted on the sync queue after w1b: each HWDGE
            # descriptor-gen costs 625ns of the serial gen pipe, and a
            # second-queue DMA would slot its gen between w1a and block 0,
            # delaying the first conv matmul by that much.
            cst_s = consts.tile([128, 3], f32)
            w2_s = cst_s[:, 0:1]
            nb_s = cst_s[:, 1:2]
            # Zeros tile for the p-state priming matmuls, zeroed on the
            # otherwise-idle DVE (ACT memzero would queue behind a 1.3us
            # LoadActFuncSet; Pool runs the framework preamble).
            z_s = consts.tile([CHID, 256], bf16)
            nc.vector.memset(z_s[:], 0.0)

            # Prime the PE p-state ramp (full clock needs ~3us since
            # pe_busy_start) with throwaway matmuls while DMAs land,
            # sized to end right as block 0 becomes readable (~4.2us).
            prime = psp.tile([CHID, TS], f32, name="prime", tag="pp")
            for _ in range(13):
                nc.tensor.matmul(out=prime[:, :256], lhsT=z_s[:, 0:CHID],
                                 rhs=z_s[:, 0:256], start=True, stop=True)
            for _ in range(2):
                nc.tensor.matmul(out=prime[:, :64], lhsT=z_s[:, 0:CHID],
                                 rhs=z_s[:, 0:64], start=True, stop=True)

            # Segment = (tile, row_off, nrows): conv output rows
            # [8*tile+row_off, +nrows) x 64 cols, n = nrows*64 columns.
            def segs_for(first, last):
                if first:
                    # Single-seg leading groups: tile 1's block cannot
                    # stall tile 0's k-loop during the cold start.
                    return [[(0, 0, 8)], [(1, 0, 8)]] + [
                        [(t, 0, 8), (t + 1, 0, 8)] for t in (2, 4, 6)]
                if last:
                    # Four equal 256-px trailing groups: each flush can
                    # only start when its whole k-loop ends, so the tail
                    # chains get conv cover from the groups after them.
                    # They share one out_g (rows 0/32/64/96) so a SINGLE
                    # strided y DMA runs after the last sigmoid -- tail
                    # descriptor-gens don't stack on the serial HWDGE.
                    return ([[(t, 0, 8), (t + 1, 0, 8)] for t in (0, 2, 4)]
                            + [[(6, 0, 4)], [(6, 4, 4)], [(7, 0, 4)],
                               [(7, 4, 4)]])
                return [[(t, 0, 8), (t + 1, 0, 8)] for t in (0, 2, 4, 6)]

            # Each group's pointwise chain is emitted right after its conv
            # matmuls: the chain contains no PE instructions (DVE -> GpSimd
            # -> ACT -> DMA), so it runs under the NEXT group's conv window
            # and only the final tiny group's chain trails the last matmul.
            def flush(pend, out_g=None, grow=0, emit=True):
                gg, rr, spans, ptiles, img = pend
                pdim = list(cst_s.ap)[0]
                for g, n, ypos in spans:
                    # g = max(psum, -SX*SW*b1[c]) * (W2[c]/4096) on DVE:
                    # per-partition scalar AP for the bias, stride-0
                    # broadcast of the 1x1 weight along the free dim.
                    nc.vector.scalar_tensor_tensor(
                        out=gg[:, g * TS:g * TS + n],
                        in0=ptiles[g][:, :n],
                        scalar=nb_s,
                        in1=AP(w2_s.tensor, w2_s.offset, [pdim, [0, n]]),
                        op0=mybir.AluOpType.max,
                        op1=mybir.AluOpType.mult,
                    )
                for g, n, ypos in spans:
                    # Channel sum on the otherwise-idle GpSimd engine.
                    nc.gpsimd.partition_all_reduce(
                        out_ap=rr[:, g * TS:g * TS + n],
                        in_ap=gg[:, g * TS:g * TS + n],
                        channels=128,
                        reduce_op=bass_isa.ReduceOp.add,
                    )
                if out_g is None:
                    out_g = obuf.tile([97, TS], f32, name="og", tag="og")
                ostr = list(out_g.ap)[0][0]
                rows = []
                for g, n, ypos in spans:
                    r = 32 * (grow + g)
                    rows.append((r, n, ypos))
                    nc.scalar.activation(
                        out=out_g[r:r + 1, :n],
                        in_=rr[r:r + 1, g * TS:g * TS + n],
                        func=mybir.ActivationFunctionType.Sigmoid,
                        bias=cst_s[r:r + 1, 2:3], scale=1.0,
                    )
                if not emit:
                    return
                # DMA APs may stride partitions (engine APs may not): one
                # strided DMA per run of equal-length rows.
                i0 = 0
                while i0 < len(rows):
                    i1 = i0 + 1
                    while (i1 < len(rows) and rows[i1][1] == rows[i0][1]
                           and rows[i1][0] == rows[i1 - 1][0] + 32):
                        i1 += 1
                    m = i1 - i0
                    r0, n, y0 = rows[i0]
                    nc.sync.dma_start(
                        out=y[img, y0:y0 + m * n],
                        in_=AP(out_g.tensor, out_g.offset + r0 * ostr,
                               [[32 * ostr, m], [1, n]]))
                    i0 = i1

            iters = [ii for _ in range(reps) for ii in range(BLOC)]
            for idx, i in enumerate(iters):
                first = idx == 0
                last = idx == len(iters) - 1
                xf = xin.tile([128, XFW], fp8, tag="x")
                pstride = list(xf.ap)[0][0]
                # Block 0 splits at the chunk-2 boundary (tile 0's first
                # conv pairs read only chunks 0-1, so the first matmul can
                # start 235ns earlier); block 1 ships alone; later blocks
                # ship in pairs -- each HWDGE gen costs 625ns of the serial
                # gen pipe, and DMA bandwidth leads the PE by 2x, so fewer
                # gens beat finer intervals.
                for t0, nb_ in ((0, 1), (1, 1), (2, 2), (4, 2), (6, 2)):
                    xi = x[i, t0]
                    nc.sync.dma_start(
                        out=xf[:, t0 * BLKW:(t0 + nb_) * BLKW],
                        in_=AP(xi.tensor, xi.offset,
                               [[BLKW, 128], [128 * BLKW, nb_], [1, BLKW]]))
                    if first and t0 == 0:
                        nc.sync.dma_start(out=w1_s[:, W1SPLIT:NPAIR],
                                          in_=w1[:, W1SPLIT:NPAIR])
                        nc.sync.dma_start(out=cst_s, in_=cst[:])

                def conv(ptile, seg, k, start, stop):
                    t, row_off, nrows = seg
                    base = t * BLKW + BASEA[k] + row_off * BW
                    rhs = AP(xf.tensor, xf.offset + base,
                             [[pstride, 128], [DELTA[k], 2],
                              [BW, nrows], [1, W]])
                    nc.tensor.matmul(
                        out=ptile[:, :nrows * W],
                        lhsT=w1_s[:, k],
                        rhs=rhs,
                        start=start,
                        stop=stop,
                        perf_mode=DR,
                    )

                groups = segs_for(first, last)
                ntail = 4 if last else 0
                out_tail = None
                for gi, grp in enumerate(groups):
                    ptiles = [ps1.tile([CHID, TS], f32, tag=f"p{g % 2}",
                                       name=f"p1_{g % 2}")
                              for g in range(len(grp))]
                    # k-outer / seg-inner: one weight load per pair
                    # serves the whole group.
                    for k in range(NPAIR):
                        for g, seg in enumerate(grp):
                            conv(ptiles[g], seg, k, k == 0, k == NPAIR - 1)
                    gg = gbuf.tile([CHID, 2 * TS], f32, tag="gg", name="gg")
                    rr = rbuf.tile([128, 2 * TS], f32, tag="rr", name="rr")
                    spans = [(g, nrows * W, t * TS + row_off * W)
                             for g, (t, row_off, nrows) in enumerate(grp)]
                    ti = gi - (len(groups) - ntail)
                    if ti < 0:
                        flush((gg, rr, spans, ptiles, i))
                        continue
                    if out_tail is None:
                        out_tail = obuf.tile([97, TS], f32, name="og",
                                             tag="og")
                    flush((gg, rr, spans, ptiles, i), out_g=out_tail,
                          grow=ti, emit=False)
                    if ti == ntail - 1:
                        # One strided DMA covers all four tail rows.
                        ostr = list(out_tail.ap)[0][0]
                        y0 = 4096 - ntail * 256
                        nc.sync.dma_start(
                            out=y[i, y0:4096],
                            in_=AP(out_tail.tensor, out_tail.offset,
                                   [[32 * ostr, ntail], [1, 256]]))
    nc.compile()
    return nc


def _prep_inputs(dino_features, W1, b1, W2, b2):
    dino_features = np.asarray(dino_features, dtype=np.float32)
    W1 = np.asarray(W1, dtype=np.float32)
    b1 = np.asarray(b1, dtype=np.float32).reshape(CHID)
    W2 = np.asarray(W2, dtype=np.float32).reshape(CHID)
    b2 = np.float32(np.asarray(b2, dtype=np.float32).reshape(()))

    # Pad, scale, cast to fp8 once, then gather overlapping row-blocks.
    xp = np.zeros((B, BW, BW, CIN), dtype=np.float32)
    xp[:, 1:H + 1, 1:W + 1, :] = dino_features * SX
    xt = np.ascontiguousarray(xp.transpose(0, 3, 1, 2)).astype(E4M3)
    xt = xt.reshape(B, NCHUNK, 128, BW, BW)
    xb = np.empty((B, NT, 128, NCHUNK, CHW), dtype=E4M3)
    for t in range(NT):
        blk = xt[:, :, :, 8 * t:8 * t + BROWS, :].reshape(
            B, NCHUNK, 128, CHW)
        xb[:, t] = blk.transpose(0, 2, 1, 3)
    xb = xb.reshape(B, NT, 128, BLKW)

    # W1 (3,3,384,128) (ky,kx,ci,co) -> [chunk, cin128, tap, cout] pairs.
    wq = (W1 * SW).astype(E4M3)
    wr = wq.transpose(2, 0, 1, 3).reshape(NCHUNK, 128, 9, CHID)
    w1p = np.zeros((128, NPAIR, 2, CHID), dtype=E4M3)
    for k, (cA, tA, cB, tB) in enumerate(PAIRS):
        if cA is not None:
            w1p[:, k, 0, :] = wr[cA, :, tA, :]
        w1p[:, k, 1, :] = wr[cB, :, tB, :]

    csth = np.empty((128, 3), dtype=np.float32)
    csth[:, 0] = W2 * DESCALE
    csth[:, 1] = -(SX * SW) * b1
    csth[:, 2] = b2 + np.float32(np.dot(W2, b1))

    in_maps = []
    for c in range(NCORES):
        in_maps.append({
            "x": np.ascontiguousarray(xb[c * BLOC:(c + 1) * BLOC]),
            "w1": w1p, "cst": csth,
        })
    return in_maps


def kernel(dino_features, W1, b1, W2, b2, _trace=False, _trace_kwargs=None):
    if "nc" not in _CACHED:
        _CACHED["nc"] = _build_bass()
    nc = _CACHED["nc"]
    in_maps = _prep_inputs(dino_features, W1, b1, W2, b2)
    res = run_bass_kernel_spmd(nc, in_maps, core_ids=list(range(NCORES)),
                               trace=_trace, **(_trace_kwargs or {}))
    _CACHED["last_results"] = res
    out = np.concatenate([res.results[c]["y"] for c in range(NCORES)], axis=0)
    # y is densely packed: flat col 64*h + w -> pixel (h, w).
    return np.ascontiguousarray(out).reshape(B, H, W, 1).astype(np.float32)
